# revision 7
# baseline (speedup 1.0000x reference)
"""Trainium2 kernel for ClusterNet forward (51x51 box-filter cluster voting).

Math (cnt cancels between the two avg_pools):
    oc   = cluster_assignments + 1e-6                      # (c,h,w)
    nn   = nn_probs[0]                                     # (l,h,w)
    out_l = sum_c (oc_c / box(oc_c)) * box(oc_c * nn_l)    # box = 51x51 zero-padded SUM

Sharding: h split across 8 cores (128 output rows each) with a 25-row halo
(zero-padded at the global edges on host). All spatial box filtering is done
on the tensor engine as banded matmuls:
  conv1 (h-direction): out[ho,w] = B1.T @ rows0 + B2.T @ rows1
  conv2 (w-direction): on PE-transposed intermediate with -25-offset column
        tiles so every 128-wide output block needs exactly 2 matmuls with the
        SAME two banded stationaries B1/B2.
"""

import sys
import numpy as np

try:
    import concourse.bass as bass
except ImportError:  # pragma: no cover
    sys.path.insert(0, "/opt/trn_rl_repo")
    import concourse.bass as bass

import ml_dtypes
from concourse import mybir
from concourse.bass_utils import run_bass_kernel_spmd
from concourse.tile import TileContext
from concourse.vector_clock import ScopedClock

BF16 = ml_dtypes.bfloat16
C, L, H, W = 8, 8, 1024, 1024
NCORES = 8
R = 25
BAND = 2 * R          # 50
RO = H // NCORES      # 128 output rows per core
RI = RO + 2 * R       # 178 input rows per core
NJ = W // 128         # 8 wo blocks
YPW = 128 * (NJ + 1)  # 1152 padded y width (25 left pad + 1024 + 103 right pad)

# Walrus in this toolchain accepts at most one sync-wait per instruction.
# After Tile scheduling, split any instruction carrying N>1 waits into N-1
# preceding same-engine wait-nops plus the original with a single wait.
_MAX_WAITS = 1
SafeTileContext = TileContext


def _split_multi_waits(nc):
    counter = [0]
    for fn in nc.m.functions:
        for bb in fn.blocks:
            new_insts = []
            changed = False
            for inst in bb.instructions:
                si = getattr(inst, "sync_info", None)
                waits = list(si.on_wait) if si and si.on_wait else []
                if len(waits) > _MAX_WAITS:
                    changed = True
                    extra, keep = waits[:-_MAX_WAITS], waits[-_MAX_WAITS:]
                    for i in range(0, len(extra), _MAX_WAITS):
                        counter[0] += 1
                        new_insts.append(
                            mybir.InstNoOp(
                                name=f"I-WSPLIT-{counter[0]}",
                                engine=inst.engine,
                                bass_nofuse=True,
                                sync_info=mybir.SyncInfo(
                                    on_wait=extra[i : i + _MAX_WAITS], on_update=[]
                                ),
                            )
                        )
                    inst.sync_info = mybir.SyncInfo(
                        on_wait=keep, on_update=list(si.on_update or [])
                    )
                new_insts.append(inst)
            if changed:
                try:
                    bb.instructions[:] = new_insts
                except TypeError:
                    bb.instructions = new_insts


def _band_matrices():
    # B1[r, m] = 1 iff m <= r <= m+50   (128x128)
    r = np.arange(128)[:, None]
    m = np.arange(128)[None, :]
    b1 = ((m <= r) & (r <= m + BAND)).astype(np.float32)
    # B2[r2, m] = 1 iff r2 <= m-78      (50x128)
    r2 = np.arange(BAND)[:, None]
    b2 = (r2 <= m - (128 - BAND)).astype(np.float32)
    return b1.astype(BF16), b2.astype(BF16)


def _build_module():
    nc = bass.Bass("TRN2", target_bir_lowering=False, debug=False, num_devices=NCORES)
    f32 = mybir.dt.float32
    bf16 = mybir.dt.bfloat16

    ocp = nc.declare_dram_parameter("oc", [C, RI, W], bf16, isOutput=False)
    nnp = nc.declare_dram_parameter("nn", [L, RI, W], bf16, isOutput=False)
    b1 = nc.declare_dram_parameter("b1", [128, 128], bf16, isOutput=False)
    b2 = nc.declare_dram_parameter("b2", [BAND, 128], bf16, isOutput=False)
    idb = nc.declare_dram_parameter("idb", [128, 128], bf16, isOutput=False)
    idf = nc.declare_dram_parameter("idf", [128, 128], f32, isOutput=False)
    outp = nc.declare_dram_parameter("out", [L, RO, W], f32, isOutput=True)

    with SafeTileContext(nc) as tc:
        import contextlib

        with contextlib.ExitStack() as ctx:
            persist = ctx.enter_context(tc.tile_pool(name="persist", bufs=1))
            jt_pool = ctx.enter_context(tc.tile_pool(name="jt", bufs=3))
            tp_pool = ctx.enter_context(tc.tile_pool(name="tp", bufs=2))
            tmp_pool = ctx.enter_context(tc.tile_pool(name="tmp", bufs=3))
            out_pool = ctx.enter_context(tc.tile_pool(name="outb", bufs=2))
            p1 = ctx.enter_context(tc.tile_pool(name="p1", bufs=2, space="PSUM"))
            pt = ctx.enter_context(tc.tile_pool(name="ptp", bufs=2, space="PSUM"))
            p2 = ctx.enter_context(tc.tile_pool(name="p2", bufs=1, space="PSUM"))

            # --- constants ---
            b1_sb = persist.tile([128, 128], bf16, tag="b1")
            b2_sb = persist.tile([BAND, 128], bf16, tag="b2")
            idb_sb = persist.tile([128, 128], bf16, tag="idb")
            idf_sb = persist.tile([128, 128], f32, tag="idf")
            nc.sync.dma_start(out=b1_sb[:], in_=b1[:])
            nc.sync.dma_start(out=b2_sb[:], in_=b2[:])
            nc.sync.dma_start(out=idb_sb[:], in_=idb[:])
            nc.sync.dma_start(out=idf_sb[:], in_=idf[:])

            # --- inputs ---
            oc0, oc1, nn0, nn1 = [], [], [], []
            for c in range(C):
                t0 = persist.tile([128, W], bf16, tag=f"oc0_{c}")
                t1 = persist.tile([BAND, W], bf16, tag=f"oc1_{c}")
                nc.sync.dma_start(out=t0[:], in_=ocp[c, 0:128, :])
                nc.sync.dma_start(out=t1[:], in_=ocp[c, 128:RI, :])
                oc0.append(t0)
                oc1.append(t1)
            for l in range(L):
                t0 = persist.tile([128, W], bf16, tag=f"nn0_{l}")
                t1 = persist.tile([BAND, W], bf16, tag=f"nn1_{l}")
                nc.sync.dma_start(out=t0[:], in_=nnp[l, 0:128, :])
                nc.sync.dma_start(out=t1[:], in_=nnp[l, 128:RI, :])
                nn0.append(t0)
                nn1.append(t1)

            # --- padded conv1-output buffers (25 zero cols left, 103 right) ---
            NYB = 3
            y_bufs = []
            for i in range(NYB):
                yb = persist.tile([128, YPW], bf16, tag=f"y{i}")
                nc.vector.memset(yb[:, 0:R], 0.0)
                nc.vector.memset(yb[:, R + W : YPW], 0.0)
                y_bufs.append(yb)
            y_idx = [0]

            def conv_pipeline(src0, src1, want_f32_box):
                """src: (128,W)+(BAND,W) bf16 input tiles -> returns psum2
                (128, NJ, 128) f32 = 2D box sums in (wo, j, ho) layout."""
                yb = y_bufs[y_idx[0] % NYB]
                y_idx[0] += 1
                # conv1 (h-direction) -> psum (128, 512) x2
                for half in range(2):
                    ps = p1.tile([128, 512], mybir.dt.float32, tag="p1")
                    sl = slice(half * 512, half * 512 + 512)
                    nc.tensor.matmul(ps[:], b1_sb[:], src0[:, sl], start=True, stop=False)
                    nc.tensor.matmul(ps[:], b2_sb[:], src1[0:BAND, sl], start=False, stop=True)
                    nc.scalar.copy(out=yb[:, R + half * 512 : R + half * 512 + 512], in_=ps[:])
                # transposes (9 x 128-col blocks of padded y)
                pst = pt.tile([128, NJ + 1, 128], mybir.dt.bfloat16, tag="pt")
                for j in range(NJ + 1):
                    nc.tensor.transpose(pst[:, j, :], yb[:, 128 * j : 128 * (j + 1)], idb_sb[:])
                tp = tp_pool.tile([128, NJ + 1, 128], mybir.dt.bfloat16, tag="tp")
                nc.vector.tensor_copy(out=tp[:], in_=pst[:])
                # conv2 (w-direction)
                # NOTE: start=True clears has_written bits for the WHOLE bank,
                # so each slice's accumulation group must run consecutively.
                ps2 = p2.tile([128, NJ, 128], mybir.dt.float32, tag="p2")
                for j in range(NJ):
                    nc.tensor.matmul(ps2[:, j, :], b1_sb[:], tp[:, j, :], start=True, stop=False)
                    nc.tensor.matmul(ps2[:, j, :], b2_sb[:], tp[0:BAND, j + 1, :], start=False, stop=True)
                return ps2

            # --- phase B: u_c = oc_c(center)/box(oc_c), in (wo, j, ho) layout ---
            u_tiles = []
            for c in range(C):
                ps2 = conv_pipeline(oc0[c], oc1[c], True)
                rb = tmp_pool.tile([128, NJ, 128], mybir.dt.float32, tag="rb")
                nc.vector.reciprocal(out=rb[:], in_=ps2[:])
                # transpose center oc rows (global rows R..R+128) per j block.
                # Matmul bases must be 32-aligned, so transpose full row
                # blocks and read the center rows via column offsets.
                psoA = pt.tile([128, NJ, 128], mybir.dt.bfloat16, tag="pt")
                psoB = pt.tile([128, NJ, BAND], mybir.dt.bfloat16, tag="pt")
                for j in range(NJ):
                    cs = slice(128 * j, 128 * (j + 1))
                    nc.tensor.transpose(psoA[:, j, :], oc0[c][:, cs], idb_sb[:])
                    nc.tensor.transpose(psoB[:, j, 0:BAND], oc1[c][0:BAND, cs], idb_sb[0:BAND, 0:BAND])
                uc = persist.tile([128, NJ, 128], mybir.dt.bfloat16, tag=f"u{c}")
                nc.vector.tensor_mul(uc[:, :, 0 : 128 - R], psoA[:, :, R:128], rb[:, :, 0 : 128 - R])
                nc.vector.tensor_mul(uc[:, :, 128 - R : 128], psoB[:, :, 0:R], rb[:, :, 128 - R : 128])
                u_tiles.append(uc)

            # --- accumulators ---
            accs = []
            for l in range(L):
                a = persist.tile([128, NJ, 128], mybir.dt.float32, tag=f"acc{l}")
                accs.append(a)

            # --- phase C: 64 channel pairs ---
            for c in range(C):
                for l in range(L):
                    j0 = jt_pool.tile([128, W], mybir.dt.bfloat16, tag="j0")
                    j1 = jt_pool.tile([BAND, W], mybir.dt.bfloat16, tag="j1")
                    nc.vector.tensor_mul(j0[:], oc0[c][:], nn0[l][:])
                    nc.vector.tensor_mul(j1[:], oc1[c][:], nn1[l][:])
                    ps2 = conv_pipeline(j0, j1, False)
                    if c == 0:
                        nc.vector.tensor_mul(accs[l][:], ps2[:], u_tiles[c][:])
                    else:
                        tmp = tmp_pool.tile([128, NJ, 128], mybir.dt.bfloat16, tag="cmb")
                        nc.vector.tensor_mul(tmp[:], ps2[:], u_tiles[c][:])
                        nc.vector.tensor_add(accs[l][:], accs[l][:], tmp[:])

            # --- phase D: transpose back to natural layout and store ---
            for l in range(L):
                ob = out_pool.tile([128, W], mybir.dt.float32, tag="ob")
                for j in range(NJ):
                    psf = pt.tile([128, 128], mybir.dt.float32, tag="pt")
                    nc.tensor.transpose(psf[:], accs[l][:, j, :], idf_sb[:])
                    nc.scalar.copy(out=ob[:, 128 * j : 128 * (j + 1)], in_=psf[:])
                nc.sync.dma_start(out=outp[l], in_=ob[:])

    _split_multi_waits(nc)
    return nc


_NC_CACHE = {}
TRACE = False
LAST_EXEC_NS = None


def kernel(cluster_assignments, nn_probs):
    global LAST_EXEC_NS
    if "nc" not in _NC_CACHE:
        _NC_CACHE["nc"] = _build_module()
    nc = _NC_CACHE["nc"]

    oc = cluster_assignments.astype(np.float32) + 1e-6
    nn = nn_probs[0].astype(np.float32)

    # pad rows by R with zeros, then slice per core
    ocz = np.zeros((C, H + 2 * R, W), np.float32)
    ocz[:, R : R + H] = oc
    nnz = np.zeros((L, H + 2 * R, W), np.float32)
    nnz[:, R : R + H] = nn
    ocz = ocz.astype(BF16)
    nnz = nnz.astype(BF16)

    b1, b2 = _band_matrices()
    idb = np.eye(128, dtype=BF16)
    idf = np.eye(128, dtype=np.float32)

    in_maps = []
    for k in range(NCORES):
        lo = RO * k  # in padded coords: rows lo .. lo+RI
        in_maps.append(
            {
                "oc": np.ascontiguousarray(ocz[:, lo : lo + RI]),
                "nn": np.ascontiguousarray(nnz[:, lo : lo + RI]),
                "b1": b1,
                "b2": b2,
                "idb": idb,
                "idf": idf,
            }
        )

    res = run_bass_kernel_spmd(nc, in_maps, list(range(NCORES)), trace=TRACE)
    LAST_EXEC_NS = res.exec_time_ns
    out = np.concatenate([res.results[k]["out"] for k in range(NCORES)], axis=1)
    return out
